# revision 8
# baseline (speedup 1.0000x reference)
"""GAT 2-layer kernel for Trainium2, 8 NeuronCores (Bass/Tile).

Strategy (graph/data parallel per the sharding hint):
  - Nodes are degree-sorted and dealt round-robin to the 8 cores; each core
    owns the edges whose dst it owns, so edge-softmax and the weighted
    aggregation are core-local.
  - Per GAT layer, two SPMD launches:
      A-launch: table build, sharded by node id - core c computes rows of
        T = X @ [W | W@al | W@ar]  (h plus per-node attention terms).
      B-launch: per 128-dst-node tile, gather source h rows with dma_gather
        (4 SWDGE queues; two overlapping int16-indexed table windows, host
        balances edges between them), compute exp(leaky_relu(el+er)) with
        denominators via activation accum, weighted-sum via broadcast
        multiply + strided reduce, normalize, bias, activation.
  - The host only routes bytes between launches (shard/gather/concat); all
    arithmetic runs on device.
"""

import os
import sys
import types
import numpy as np

sys.path.insert(0, "/opt/trn_rl_repo")

N = 50000
E = 800000
CIN = 128
NCORES = 8
NSH = N // NCORES            # 6250 nodes per core
TB = (NSH + 127) // 128      # 49 dst tiles per core
NSHPAD = TB * 128            # 6272
NPAD = NCORES * NSHPAD       # 50176 table rows (= 392 tiles of 128)
TApc = TB                    # table tiles per core in the A-launch (49)
NEG = 0.2
WIN = min(32768, NPAD)       # int16 index window
HB0 = NPAD - WIN             # 17408: start of table window B
F32 = np.float32

_results_log = []            # BassKernelResults per launch (timing for test.py)


def _install_trace_support():
    """Register the NTFF profile hook this image's antenv lacks, and make
    artifact upload a no-op (no bucket here). Lets BASS_TRACE/trace=True
    report exec_time_ns."""
    try:
        from antenv.axon_hooks import get_axon_ntff_profile_hook  # noqa: F401
        return
    except ImportError:
        pass
    try:
        import trn_agent_boot.trn_boot as tb
        hook = tb._ntff_profile_via_ctypes("/opt/axon/libaxon_pjrt.so")
        mod = types.ModuleType("antenv.axon_hooks")
        state = {"h": hook}
        mod.get_axon_ntff_profile_hook = lambda: state["h"]
        mod.set_axon_ntff_profile_hook = lambda h: state.__setitem__("h", h)
        sys.modules["antenv.axon_hooks"] = mod
        import antenv
        antenv.axon_hooks = mod
        from concourse import bass_utils as bu
        orig = bu.upload_artifacts

        def safe_upload(tmpdir):
            try:
                return orig(tmpdir)
            except Exception:
                return tmpdir
        bu.upload_artifacts = safe_upload
    except Exception:
        pass


_install_trace_support()


# --------------------------------------------------------------------------
# device programs
# --------------------------------------------------------------------------

def _build_tab_launch(tcols):
    """A-launch: core-sharded table build T_shard = XT_shard @ WE."""
    from concourse import mybir, tile, bacc

    f32 = mybir.dt.float32
    nc = bacc.Bacc("TRN2", target_bir_lowering=False, debug=False,
                   enable_asserts=False)
    XT = nc.dram_tensor("xt", [TApc * 128, 128], f32, kind="ExternalInput")
    WE = nc.dram_tensor("we", [CIN, tcols], f32, kind="ExternalInput")
    TAB = nc.dram_tensor("tab", [NSHPAD, tcols], f32, kind="ExternalOutput")

    with tile.TileContext(nc) as tc:
        with tc.tile_pool(name="c", bufs=1) as cpool, \
             tc.tile_pool(name="pa", bufs=4) as pa, \
             tc.tile_pool(name="ps", bufs=4, space="PSUM") as pp:
            we_t = cpool.tile([CIN, tcols], f32)
            nc.sync.dma_start(we_t[:], WE[:, :])
            for j in range(TApc):
                xt = pa.tile([CIN, 128], f32, tag="xt")
                nc.sync.dma_start(xt[:], XT[j * 128:(j + 1) * 128, :])
                ps = pp.tile([128, tcols], f32, tag="ps")
                nc.tensor.matmul(out=ps[:], lhsT=xt[:], rhs=we_t[:],
                                 start=True, stop=True)
                ot = pa.tile([128, tcols], f32, tag="ot")
                nc.vector.tensor_copy(out=ot[:], in_=ps[:])
                nc.sync.dma_start(TAB[j * 128:(j + 1) * 128, :], ot[:])
    nc.compile()
    return nc


def _build_agg_launch(layer, heads, d, kas, kbs):
    """B-launch: gather + attention + weighted aggregation for own shard."""
    from concourse import mybir, tile, bacc

    cout = heads * d
    f32 = mybir.dt.float32
    i16 = mybir.dt.int16
    AT = mybir.ActivationFunctionType
    OP = mybir.AluOpType

    k2 = [int(kas[t] + kbs[t]) for t in range(TB)]
    ko2 = np.concatenate([[0], np.cumsum(k2)[:-1]]).astype(int)
    k2tot = int(sum(k2))
    cw = 8 * k2tot  # wrapped idx columns

    nc = bacc.Bacc("TRN2", target_bir_lowering=False, debug=False,
                   enable_asserts=False, num_swdge_queues=4)
    HT = nc.dram_tensor("ht", [NPAD, cout], f32, kind="ExternalInput")
    IX = nc.dram_tensor("ix", [128, cw], i16, kind="ExternalInput")
    EL = nc.dram_tensor("el", [128, heads * k2tot], f32, kind="ExternalInput")
    ER = nc.dram_tensor("er", [128, TB * heads], f32, kind="ExternalInput")
    BI = nc.dram_tensor("bi", [128, cout], f32, kind="ExternalInput")
    if layer == 2:
        H1S = nc.dram_tensor("h1s", [NSHPAD, 128], f32, kind="ExternalInput")
    OUT = nc.dram_tensor("out", [NSHPAD, cout], f32, kind="ExternalOutput")

    qrr = [0]  # round-robin SWDGE queue

    with tile.TileContext(nc) as tc:
        with tc.tile_pool(name="c", bufs=1) as cpool, \
             tc.tile_pool(name="pb", bufs=2) as pb, \
             tc.tile_pool(name="sm", bufs=4) as sm:
            ix_t = cpool.tile([128, cw], i16)
            nc.sync.dma_start(ix_t[:], IX[:, :])
            el_t = cpool.tile([128, heads * k2tot], f32)
            nc.sync.dma_start(el_t[:], EL[:, :])
            er_t = cpool.tile([128, TB * heads], f32)
            nc.sync.dma_start(er_t[:], ER[:, :])
            bi_t = cpool.tile([128, cout], f32)
            nc.sync.dma_start(bi_t[:], BI[:, :])

            for t in range(TB):
                kt = k2[t]
                ko = int(ko2[t])
                g = pb.tile([128, kt * cout], f32, tag="g")
                # gathers: window A covers slot cols [0,ka), B covers [ka,kt)
                for half, base in ((0, 0), (1, int(kas[t]))):
                    nblk_all = int(kas[t]) if half == 0 else int(kbs[t])
                    src_ap = HT[:, :] if half == 0 else HT[HB0:, :]
                    b0 = 0
                    while b0 < nblk_all:
                        nb = min(8, nblk_all - b0)
                        kcol = base + b0
                        cbase = 8 * (ko + kcol)
                        gslice = g[:, kcol * cout:(kcol + nb) * cout]
                        nc.gpsimd.dma_gather(
                            out_ap=gslice.rearrange("p (b e) -> p b e",
                                                    e=cout),
                            in_ap=src_ap,
                            idxs_ap=ix_t[:, cbase:cbase + 8 * nb],
                            num_idxs=nb * 128,
                            num_idxs_reg=nb * 128,
                            elem_size=cout,
                            queue_num=qrr[0] % 4)
                        qrr[0] += 1
                        b0 += nb

                # attention
                ex = sm.tile([128, heads * kt], f32, tag="ex")
                den = sm.tile([128, heads], f32, tag="den")
                for hh in range(heads):
                    e0 = sm.tile([128, kt], f32, tag="e0")
                    nc.vector.tensor_scalar(
                        out=e0[:], in0=el_t[:, hh * k2tot + ko:
                                            hh * k2tot + ko + kt],
                        scalar1=er_t[:, t * heads + hh:t * heads + hh + 1],
                        scalar2=None, op0=OP.add)
                    e1 = sm.tile([128, kt], f32, tag="e1")
                    nc.vector.tensor_scalar(out=e1[:], in0=e0[:], scalar1=NEG,
                                            scalar2=None, op0=OP.mult)
                    nc.vector.tensor_tensor(out=e0[:], in0=e0[:], in1=e1[:],
                                            op=OP.max)
                    nc.scalar.activation(
                        out=ex[:, hh * kt:(hh + 1) * kt], in_=e0[:],
                        func=AT.Exp, accum_out=den[:, hh:hh + 1])

                rd = sm.tile([128, heads], f32, tag="rd")
                nc.vector.tensor_scalar(out=rd[:], in0=den[:], scalar1=1e-12,
                                        scalar2=None, op0=OP.max)
                nc.vector.reciprocal(out=rd[:], in_=rd[:])

                # messages + reduce over slots
                msg = pb.tile([128, kt * cout], f32, tag="msg")
                gv = g[:].rearrange("p (k c) -> p k c", c=cout)
                mv = msg[:].rearrange("p (k c) -> p k c", c=cout)
                for hh in range(heads):
                    nc.vector.tensor_tensor(
                        out=mv[:, :, hh * d:(hh + 1) * d],
                        in0=gv[:, :, hh * d:(hh + 1) * d],
                        in1=ex[:, hh * kt:(hh + 1) * kt].to_broadcast(
                            [128, kt, d]),
                        op=OP.mult)
                num = sm.tile([128, cout], f32, tag="num")
                nc.vector.tensor_reduce(
                    out=num[:],
                    in_=msg[:].rearrange("p (k c) -> p c k", c=cout),
                    axis=mybir.AxisListType.X, op=OP.add)

                # normalize + bias + layer activation / final combine
                o = sm.tile([128, cout], f32, tag="o")
                for hh in range(heads):
                    nc.vector.tensor_scalar(
                        out=o[:, hh * d:(hh + 1) * d],
                        in0=num[:, hh * d:(hh + 1) * d],
                        scalar1=rd[:, hh:hh + 1], scalar2=None, op0=OP.mult)
                nc.vector.tensor_tensor(out=o[:], in0=o[:], in1=bi_t[:],
                                        op=OP.add)
                if layer == 1:
                    # elu(x) = max(x, exp(min(x,0)) - 1)
                    mz = sm.tile([128, cout], f32, tag="mz")
                    nc.vector.tensor_scalar(out=mz[:], in0=o[:], scalar1=0.0,
                                            scalar2=None, op0=OP.min)
                    nc.scalar.activation(out=mz[:], in_=mz[:], func=AT.Exp)
                    nc.vector.tensor_scalar(out=mz[:], in0=mz[:], scalar1=1.0,
                                            scalar2=None, op0=OP.subtract)
                    nc.vector.tensor_tensor(out=o[:], in0=o[:], in1=mz[:],
                                            op=OP.max)
                else:
                    # final = 0.25*(h1_h0 + h1_h1) + 0.5*(gat2 + b2)
                    h1t = pb.tile([128, 128], f32, tag="h1t")
                    nc.sync.dma_start(h1t[:], H1S[t * 128:(t + 1) * 128, :])
                    hm = sm.tile([128, d], f32, tag="hm")
                    nc.vector.tensor_tensor(out=hm[:], in0=h1t[:, 0:d],
                                            in1=h1t[:, d:2 * d], op=OP.add)
                    nc.vector.tensor_scalar(out=hm[:], in0=hm[:], scalar1=0.25,
                                            scalar2=None, op0=OP.mult)
                    nc.vector.tensor_scalar(out=o[:], in0=o[:], scalar1=0.5,
                                            scalar2=None, op0=OP.mult)
                    nc.vector.tensor_tensor(out=o[:], in0=o[:], in1=hm[:],
                                            op=OP.add)
                nc.sync.dma_start(OUT[t * 128:(t + 1) * 128, :], o[:])
    nc.compile()
    return nc


# --------------------------------------------------------------------------
# host-side graph prep
# --------------------------------------------------------------------------

def _prep_graph(src, dst):
    """Degree-sorted round-robin sharding + balanced two-window CSR."""
    deg = np.bincount(dst, minlength=N)
    ranks = np.argsort(-deg, kind="stable").astype(np.int64)
    pos = np.empty(N, np.int64)
    pos[ranks] = np.arange(N)
    ec = (pos[dst] % NCORES).astype(np.int64)
    ej = (pos[dst] // NCORES).astype(np.int64)
    src = src.astype(np.int64)

    # per-core per-tile forced/flex counts (window A = [0,WIN), B = [HB0,NPAD))
    per_core = []
    kas = np.ones(TB, np.int64)
    kbs = np.zeros(TB, np.int64)
    for c in range(NCORES):
        m = ec == c
        js, ss = ej[m], src[m]
        order = np.argsort(js * (2 * N) + ss, kind="stable")  # by node, by src
        js, ss = js[order], ss[order]
        cnt = np.bincount(js, minlength=NSHPAD)
        loF = np.bincount(js, weights=(ss < HB0), minlength=NSHPAD)
        hiF = np.bincount(js, weights=(ss >= WIN), minlength=NSHPAD)
        d_t = cnt.reshape(TB, 128)
        lo_t = loF.astype(np.int64).reshape(TB, 128)
        hi_t = hiF.astype(np.int64).reshape(TB, 128)
        fl_t = d_t - lo_t - hi_t
        for t in range(TB):
            dd, lo, hi, fl = d_t[t], lo_t[t], hi_t[t], fl_t[t]
            best_ka, best_sum = None, None
            for KA in range(int(lo.max()), int((lo + fl).max()) + 1):
                KB = int(np.maximum(dd - np.minimum(lo + fl, KA), hi).max())
                if best_sum is None or KA + KB < best_sum:
                    best_ka, best_sum = KA, KA + KB
            if best_ka is None:
                best_ka, best_sum = 0, 0
            kas[t] = max(kas[t], best_ka)
            kbs[t] = max(kbs[t], best_sum - best_ka)
        per_core.append((js, ss, cnt))
    k2 = kas + kbs
    ko2 = np.concatenate([[0], np.cumsum(k2)[:-1]]).astype(np.int64)
    k2tot = int(k2.sum())

    # per-core slot tables: srcidx[p, global_slot_col] (node-id space), with
    # window assignment honoring shared (kas, kbs) capacities.
    slot_src = []          # int64 [NCORES][128, k2tot], -1 = pad
    for c in range(NCORES):
        js, ss, cnt = per_core[c]
        starts = np.concatenate([[0], np.cumsum(cnt)[:-1]])
        tile_id = js // 128
        p = js % 128
        # within-node edge order: A-edges first, then B-edges. A-count per
        # node: nA = clip(lo + flex_used), where flex_used fills A up to kas.
        lo_e = (ss < HB0)
        hi_e = (ss >= WIN)
        node_lo = np.bincount(js, weights=lo_e, minlength=NSHPAD).astype(np.int64)
        node_hi = np.bincount(js, weights=hi_e, minlength=NSHPAD).astype(np.int64)
        node_fl = cnt - node_lo - node_hi
        node_ka = kas[np.arange(NSHPAD) // 128]
        node_kb = kbs[np.arange(NSHPAD) // 128]
        # A gets: all lo + as many flex as fit s.t. B load fits kb
        nA = np.minimum(node_lo + node_fl, node_ka)
        nA = np.maximum(nA, cnt - node_kb)  # push overflow into A if B full
        # order edges within node: lo first, flex, hi (sorted by src works:
        # lo are smallest, hi largest, flex middle) - already sorted by src.
        eidx = np.arange(len(js))
        within = eidx - starts[js]           # 0..d-1, sorted by src
        isA = within < nA[js]
        colA = within
        colB = kas[tile_id] + (within - nA[js])
        col = np.where(isA, colA, colB)
        sidx = np.where(isA, ss, ss - HB0)
        arr = np.full((128, k2tot), -1, np.int64)
        arr[p, ko2[tile_id] + col] = sidx
        slot_src.append(arr)

    return ranks, kas, kbs, k2, ko2, k2tot, slot_src


def _wrap_idx(slot_src, kas, k2, ko2, k2tot):
    """Build the wrapped int16 index array [128, 8*k2tot] for dma_gather.
    Index j of an instruction lives at [j%16, j//16], replicated x8.
    Instructions cover <=8 slot columns; cbase for slot col k is 8*(ko+k),
    and within an instruction of nb cols starting at k0, src(p,k) sits at
    wrapped[(p%16), 8*(ko+k0) + (k-k0)*8 + p//16]."""
    cw = 8 * k2tot
    out = np.zeros((16, cw), np.int16)
    p = np.arange(128)
    for t in range(len(k2)):
        ko = int(ko2[t])
        for half, base, nall in ((0, 0, int(kas[t])),
                                 (1, int(kas[t]), int(k2[t] - kas[t]))):
            b0 = 0
            while b0 < nall:
                nb = min(8, nall - b0)
                k0 = base + b0
                for kk in range(nb):
                    vals = slot_src[:, ko + k0 + kk]  # [128]
                    vals = np.where(vals < 0, 0, vals)  # pad -> row 0
                    out[p % 16, 8 * (ko + k0) + kk * 8 + p // 16] = vals
                b0 += nb
    return np.tile(out, (8, 1))


def _xt_shard(x, c):
    """Host-transposed table-build input for core c: [6272, 128] where rows
    j*128+d hold X^T tile (c*49+j)."""
    lo = c * NSHPAD
    xp = np.zeros((NSHPAD, CIN), F32)
    hi = min(N, lo + NSHPAD)
    if hi > lo:
        xp[:hi - lo] = x[lo:hi]
    return np.ascontiguousarray(
        xp.reshape(TB, 128, CIN).transpose(0, 2, 1)).reshape(TB * 128, CIN)


def _run(nc, in_maps):
    from concourse.bass_utils import run_bass_kernel_spmd
    trace = bool(os.environ.get("GAT_TRACE"))
    res = run_bass_kernel_spmd(nc, in_maps, list(range(NCORES)), trace=trace)
    _results_log.append(res)
    return res.results


def _wext(W, al, ar, heads, d):
    A = np.zeros((heads * d, heads), F32)
    R = np.zeros((heads * d, heads), F32)
    for h in range(heads):
        A[h * d:(h + 1) * d, h] = al[h]
        R[h * d:(h + 1) * d, h] = ar[h]
    return np.ascontiguousarray(np.hstack([W, W @ A, W @ R]).astype(F32))


_cache = {}


def kernel(feature, src, dst, W1, al1, ar1, b1, W2, al2, ar2, b2):
    feature = np.asarray(feature, F32)
    src = np.asarray(src, np.int32)
    dst = np.asarray(dst, np.int32)
    W1, al1, ar1, b1 = (np.asarray(a, F32) for a in (W1, al1, ar1, b1))
    W2, al2, ar2, b2 = (np.asarray(a, F32) for a in (W2, al2, ar2, b2))

    ranks, kas, kbs, k2, ko2, k2tot, slot_src = _prep_graph(src, dst)
    key = tuple(k2)
    if key not in _cache:
        _cache[key] = (
            _build_tab_launch(132),
            _build_tab_launch(66),
            _build_agg_launch(1, 2, 64, kas, kbs),
            _build_agg_launch(2, 1, 64, kas, kbs),
        )
    nc_t1, nc_t2, nc_b1, nc_b2 = _cache[key]

    idxw = [_wrap_idx(s, kas, k2, ko2, k2tot) for s in slot_src]

    # dst global node id per (core, partition, tile) for er routing
    ids = np.full((NCORES, NSHPAD), -1, np.int64)
    i = np.arange(N)
    ids[i % NCORES, i // NCORES] = ranks[i]

    def layer(lnum, x, heads, d, W, al, ar, b, nc_tab, nc_agg,
              h1_shards=None):
        cout = heads * d
        We = _wext(W, al, ar, heads, d)
        res_t = _run(nc_tab, [dict(xt=_xt_shard(x, c), we=We)
                              for c in range(NCORES)])
        tfull = np.concatenate([np.asarray(res_t[c]["tab"])
                                for c in range(NCORES)], 0)  # [NPAD, tcols]
        ht = np.ascontiguousarray(tfull[:, :cout])
        el_nodes = tfull[:, cout:cout + heads]          # [NPAD, heads]
        er_nodes = tfull[:, cout + heads:cout + 2 * heads]
        bi = np.ascontiguousarray(np.tile(b[None, :], (128, 1)).astype(F32))

        in_maps = []
        for c in range(NCORES):
            ss = slot_src[c]                 # [128, k2tot] window-based idx
            pad = ss < 0
            # recover node-id for el lookup: A slots hold src, B hold src-HB0
            colhalfB = np.zeros(k2tot, bool)
            for t in range(TB):
                colhalfB[ko2[t] + kas[t]:ko2[t] + k2[t]] = True
            nid = np.where(colhalfB[None, :], ss + HB0, ss)
            nid = np.where(pad, 0, nid)
            el = el_nodes[nid]               # [128, k2tot, heads]
            el = np.where(pad[:, :, None], -1e30, el)
            el = np.ascontiguousarray(
                el.transpose(0, 2, 1).reshape(128, -1))  # head-major
            did = ids[c]                     # [NSHPAD]
            er = np.where(did[:, None] >= 0, er_nodes[np.maximum(did, 0)], 0.0)
            er = np.ascontiguousarray(
                er.reshape(TB, 128, heads).transpose(1, 0, 2).reshape(128, -1)
            ).astype(F32)                    # [128, TB*heads]
            m = dict(ht=ht, ix=idxw[c], el=el.astype(F32), er=er, bi=bi)
            if lnum == 2:
                m["h1s"] = h1_shards[c]
            in_maps.append(m)
        res = _run(nc_agg, in_maps)
        return [np.asarray(res[c]["out"]) for c in range(NCORES)]

    h1_shards = layer(1, feature, 2, 64, W1, al1, ar1, b1, nc_t1, nc_b1)
    h1_full = np.empty((N, 128), F32)
    j = np.arange(NSH)
    for c in range(NCORES):
        h1_full[ranks[j * NCORES + c]] = h1_shards[c][:NSH]

    out_shards = layer(2, h1_full, 1, 64, W2, al2, ar2, b2, nc_t2, nc_b2,
                       h1_shards)
    out = np.empty((N, 64), F32)
    for c in range(NCORES):
        out[ranks[j * NCORES + c]] = out_shards[c][:NSH]
    return out
